# revision 17
# baseline (speedup 1.0000x reference)
"""Trainium2 Bass kernel for octonion causal self-attention (v3).

Sharding: 8 cores = 4 batches x 2 head-groups. Core c handles batch b=c//2 and
head-group g=c%2 (octonion output components 4g..4g+3 = heads 8g..8g+7).
Each core computes q/k/v projections for its components from the full x[b],
RoPE, causal attention for its 8 heads, and a fused (head-mixer @ wo) partial
projection. The host sums the two partials per batch and transposes.

Key design points:
- q/k projections run as fp8(e4m3) DoubleRow matmuls (2 k-tiles per
  instruction): ternary weights are EXACT +-1 in fp8; x is pre-quantized to
  e4m3 on the host. The score scale s_q*s_k/sqrt(D) is folded into the
  q-side RoPE tables, so fp8 weights stay exact. (Validated: rel err 7.4e-3.)
- v projection and everything downstream is bf16 (exact ternary +-1 weights;
  s_v*s_o and mixer beta folded into host-precomputed combined weights).
- The octonion head-mixer is folded into wo on the host:
  cw[ft][d,j,f] = sum_i wm[i][d,j,e] wo[ft][e,i,f] (exact fp32 precompute),
  removing the whole mixer phase.
- Attention computes S TRANSPOSED (k on partitions, q on free): exp output IS
  the PV moving operand (no PE transposes, no 288 PSUM->SBUF copies). The
  softmax denominator comes from ones-vector matmuls accumulated on PE (same
  cycle count as the removed transposes) and 1/l is applied to y during its
  single PSUM->SBUF copy via a partition-broadcast multiply.
- Causal tightening: diagonal S^T tiles only compute the valid q-column
  suffix; masking is one [128,128] triangular block add on gpsimd.
- RoPE via evens-first layout + SBUF-to-SBUF DMA half-partition swap.
- V resident in SBUF; all weights loaded once.
"""

import math
from contextlib import ExitStack

import numpy as np
import ml_dtypes

B, T, C, H, D = 4, 1024, 2048, 16, 128
C8 = C // 8  # 256
NCORES = 8
P = 128
NEG = -1.0e30

BF = ml_dtypes.bfloat16
F8 = ml_dtypes.float8_e4m3


# ---------------- octonion tables (matches reference) ----------------
def _cd_conj(a):
    n = a.shape[0]
    if n == 1:
        return a
    h = n // 2
    return np.concatenate([_cd_conj(a[:h]), -a[h:]])


def _cd_mul(a, b):
    n = a.shape[0]
    if n == 1:
        return a * b
    h = n // 2
    a1, a2 = a[:h], a[h:]
    c1, c2 = b[:h], b[h:]
    return np.concatenate(
        [
            _cd_mul(a1, c1) - _cd_mul(_cd_conj(c2), a2),
            _cd_mul(c2, a1) + _cd_mul(a2, _cd_conj(c1)),
        ]
    )


def _octonion_tables():
    signs = np.zeros((8, 8), dtype=np.float32)
    widx = np.zeros((8, 8), dtype=np.int32)
    for i in range(8):
        for j in range(8):
            ei = np.zeros(8)
            ei[i] = 1.0
            ej = np.zeros(8)
            ej[j] = 1.0
            p = _cd_mul(ei, ej)
            k = int(np.argmax(np.abs(p)))
            signs[i, j] = np.sign(p[k])
            widx[i, j] = k
    return signs, widx


SIGNS, WIDX = _octonion_tables()


def _ternary_quantize(W: np.ndarray) -> np.ndarray:
    """Replicates reference ternary_ste forward pass bit-exactly (jnp on CPU)."""
    import jax
    import jax.numpy as jnp

    with jax.default_device(jax.devices("cpu")[0]):
        Wj = jnp.asarray(W)
        s = jnp.mean(jnp.abs(Wj), axis=(-2, -1), keepdims=True) + 1e-8
        Wq = jnp.clip(jnp.round(Wj / s), -1.0, 1.0) * s
        return np.asarray(Wq)


def _ternary_parts(W: np.ndarray):
    """Exact ternary {-1,0,1} (same rounding as reference) + per-matrix scale."""
    import jax
    import jax.numpy as jnp

    with jax.default_device(jax.devices("cpu")[0]):
        Wj = jnp.asarray(np.asarray(W, dtype=np.float32))
        s = jnp.mean(jnp.abs(Wj), axis=(-2, -1), keepdims=True) + 1e-8
        tern = jnp.clip(jnp.round(Wj / s), -1.0, 1.0)
        return np.asarray(tern, dtype=np.float32), np.asarray(s, dtype=np.float32)


def _signed_full(Wt: np.ndarray, i: int) -> np.ndarray:
    """[2048, 256] block column for octonion output component i:
    rows j*256:(j+1)*256 = SIGNS[i,j] * Wt[i^j]."""
    out = np.empty((C, C8), dtype=np.float32)
    for j in range(8):
        out[j * C8 : (j + 1) * C8, :] = SIGNS[i, j] * Wt[i ^ j]
    return out


_EVENS_FIRST = np.concatenate([np.arange(0, D, 2), np.arange(1, D, 2)])

_SHARED_CACHE = {}


def _prep_shared(inputs: dict):
    """Input-independent-of-core prep (ternary parts, combined wo weights)."""
    key = id(inputs.get("x"))
    hit = _SHARED_CACHE.get(key)
    if hit is not None:
        return hit

    tq, sq = _ternary_parts(inputs["wq"])
    tk, sk = _ternary_parts(inputs["wk"])
    tv, sv = _ternary_parts(inputs["wv"])
    to, so = _ternary_parts(inputs["wo"])
    cq = np.float32(sq.mean() * sk.mean() / math.sqrt(D))
    cm = np.float32(sv.mean() * so.mean())

    mixer_W = np.asarray(inputs["mixer_W"], dtype=np.float32)
    mixer_beta = np.asarray(inputs["mixer_beta"], dtype=np.float32)

    # mixer weights with beta and scales folded: wm[i][d, j, e]
    wm = np.empty((8, P, 8, P), dtype=np.float32)
    for i in range(8):
        for j in range(8):
            wm[i, :, j, :] = (SIGNS[i, j] * mixer_W[i ^ j]) * (mixer_beta[None, :] * cm)

    # combined (mixer @ wo) per group: cw[g][ft][d, j, f]
    cw = {}
    for g in range(2):
        wo = np.empty((16, P, 8, P), dtype=np.float32)
        for ft in range(16):
            i_o, fh = ft // 2, ft % 2
            for kt in range(8):
                j = 4 * g + kt // 2
                dloc = kt % 2
                blk = SIGNS[i_o, j] * to[i_o ^ j]
                wo[ft, :, kt, :] = blk[
                    dloc * P : (dloc + 1) * P, fh * P : (fh + 1) * P
                ]
        cw[g] = np.einsum("idje,teif->tdjf", wm, wo, optimize=True).astype(BF)

    shared = {
        "tq": tq, "tk": tk, "tv": tv, "cq": cq, "cw": cw,
    }
    _SHARED_CACHE.clear()
    _SHARED_CACHE[key] = shared
    return shared


def _prep_core_inputs(inputs: dict, b: int, g: int):
    x = inputs["x"]
    fc, fs = (np.asarray(inputs["freqs_cos"], dtype=np.float32),
              np.asarray(inputs["freqs_sin"], dtype=np.float32))
    sh = _prep_shared(inputs)
    tq, tk, tv, cq = sh["tq"], sh["tk"], sh["tv"], sh["cq"]

    m = {}
    xTf = np.ascontiguousarray(np.asarray(x[b], dtype=np.float32).T)  # [2048,1024]
    m["xT"] = xTf.reshape(16, P, T).astype(BF)
    # fp8 x, packed in ct-pairs: [ce, p, u, t]
    x8 = xTf.astype(F8)
    m["xT8"] = np.ascontiguousarray(
        x8.reshape(8, 2, P, T).transpose(0, 2, 1, 3)
    )

    # q/k fp8 ternary weights, evens-first outputs, packed ct-pairs:
    # [qk, li, dh, c_p, ce, u, d2]
    wqk8 = np.empty((2, 4, 2, P, 8, 2, P), dtype=F8)
    for qk, Wt in enumerate((tq, tk)):
        for li in range(4):
            i = 4 * g + li
            Bf = _signed_full(Wt, i)  # [2048, 256], +-1
            for dh in range(2):
                Bh = Bf[:, dh * D : (dh + 1) * D][:, _EVENS_FIRST]  # [2048,128]
                wqk8[qk, li, dh] = (
                    Bh.reshape(8, 2, P, P).transpose(2, 0, 1, 3).astype(F8)
                )
    m["wqk8"] = wqk8

    # v weights: [lp, ct, c_p, dcol], ternary exact +-1 bf16
    wv = np.empty((2, 16, P, 512), dtype=np.float32)
    for lp in range(2):
        B2 = np.concatenate(
            [_signed_full(tv, 4 * g + 2 * lp + u) for u in range(2)], axis=1
        )
        wv[lp] = B2.reshape(16, P, 512)
    m["wv"] = wv.astype(BF)

    m["cw"] = sh["cw"][g]

    # RoPE tables, evens-first layout: rows 0..63 even dims, 64..127 odd.
    # rope(q')[p] = q'[p]*cos[p] + q'[p xor 64]*sin[p]; q side carries cq.
    cosP = np.ascontiguousarray(fc.T)  # [64, 1024]
    sinP = np.ascontiguousarray(fs.T)
    cosd = np.concatenate([cosP, cosP], axis=0)
    sind = np.concatenate([-sinP, sinP], axis=0)
    m["cosq"] = (cosd * cq).astype(BF)
    m["sinq"] = (sind * cq).astype(BF)
    m["cosk"] = cosd.astype(BF)
    m["sink"] = sind.astype(BF)

    m["ones"] = np.ones((P, 1), dtype=np.float32).astype(BF)

    # S^T triangular 0/1 mask block (applied multiplicatively to exp(S^T) in
    # SBUF): rows k, cols q; keep k <= q (f >= p). Unmasked scores are O(1)
    # because cq is folded into the q RoPE tables, so exp cannot overflow.
    pidx = np.arange(P)[:, None]
    fidx = np.arange(P)[None, :]
    m["mask1"] = np.where(fidx >= pidx, 1.0, 0.0).astype(np.float32).astype(BF)
    return m


# ---------------- device program ----------------
_NC_CACHE = {}


def _build_nc(repeat: int = 1):
    key = (repeat,)
    if key in _NC_CACHE:
        return _NC_CACHE[key]

    import concourse.mybir as mybir
    import concourse.tile as tile
    from concourse import bacc

    dt = mybir.dt
    ALU = mybir.AluOpType
    AF = mybir.ActivationFunctionType
    f32, bf16, f8 = dt.float32, dt.bfloat16, dt.float8e4
    DR = mybir.MatmulPerfMode.DoubleRow

    nc = bacc.Bacc("TRN2", target_bir_lowering=False)

    xT = nc.declare_dram_parameter("xT", [16, P, T], bf16, isOutput=False)
    xT8 = nc.declare_dram_parameter("xT8", [8, P, 2, T], f8, isOutput=False)
    wqk8 = nc.declare_dram_parameter(
        "wqk8", [2, 4, 2, P, 8, 2, P], f8, isOutput=False
    )
    wv = nc.declare_dram_parameter("wv", [2, 16, P, 512], bf16, isOutput=False)
    cw = nc.declare_dram_parameter("cw", [16, P, 8, P], bf16, isOutput=False)
    cosq = nc.declare_dram_parameter("cosq", [P, T], bf16, isOutput=False)
    sinq = nc.declare_dram_parameter("sinq", [P, T], bf16, isOutput=False)
    cosk = nc.declare_dram_parameter("cosk", [P, T], bf16, isOutput=False)
    sink = nc.declare_dram_parameter("sink", [P, T], bf16, isOutput=False)
    onesp = nc.declare_dram_parameter("ones", [P, 1], bf16, isOutput=False)
    mask1p = nc.declare_dram_parameter("mask1", [P, P], bf16, isOutput=False)
    outT = nc.declare_dram_parameter("outT", [C, T], f32, isOutput=True)

    DMA_ENGS = ["sync", "gpsimd", "scalar"]

    def deng(i):
        return getattr(nc, DMA_ENGS[i % 3])

    def copy_op(i, out, in_):
        # PSUM is only reachable from DVE and Activation (not gpsimd)
        if i % 2 == 0:
            nc.vector.tensor_copy(out=out, in_=in_)
        else:
            nc.scalar.copy(out=out, in_=in_)

    with tile.TileContext(nc) as tc, ExitStack() as ctx:
        cst = ctx.enter_context(tc.tile_pool(name="cst", bufs=1))
        rec_pool = ctx.enter_context(tc.tile_pool(name="recp", bufs=4))
        stage_pool = ctx.enter_context(tc.tile_pool(name="stagep", bufs=3))
        ps_proj = ctx.enter_context(tc.tile_pool(name="psproj", bufs=2, space="PSUM"))
        ps_s = ctx.enter_context(tc.tile_pool(name="pss", bufs=3, space="PSUM"))
        ps_l = ctx.enter_context(tc.tile_pool(name="psl", bufs=1, space="PSUM"))
        ps_y = ctx.enter_context(tc.tile_pool(name="psy", bufs=2, space="PSUM"))

        for _rep in range(repeat):
            qks_cm = tc.tile_pool(name="qks", bufs=1)
            qks = qks_cm.__enter__()
            qT_h = [qks.tile([P, T], bf16, tag=f"qT{i}", name=f"qTh{i}") for i in range(8)]
            kT_h = [qks.tile([P, T], bf16, tag=f"kT{i}", name=f"kTh{i}") for i in range(8)]
            vsb_cm = tc.tile_pool(name="vsb", bufs=1)
            vsb = vsb_cm.__enter__()
            v_sb = {}
            for lp in range(2):
                for tt in range(8):
                    v_sb[lp, tt] = vsb.tile(
                        [P, 512], bf16, tag=f"v{lp}_{tt}", name=f"vsb{lp}_{tt}"
                    )
            xp_cm = tc.tile_pool(name="xp", bufs=1)
            xp = xp_cm.__enter__()
            wv_cm = tc.tile_pool(name="wvp", bufs=32)
            wv_pool = wv_cm.__enter__()

            # x8 first (QK starts on it), then consts, then x, wv
            x_t = [xp.tile([P, T], bf16, tag=f"xT{i}", name=f"xt{i}") for i in range(16)]
            x8_t = [
                xp.tile([P, 2, T], f8, tag=f"x8T{i}", name=f"x8t{i}") for i in range(8)
            ]
            nd = 0
            for ce in range(8):
                deng(nd).dma_start(x8_t[ce][:], xT8[ce])
                nd += 1
            cosq_sb = cst.tile([P, T], bf16, tag="cosq")
            sinq_sb = cst.tile([P, T], bf16, tag="sinq")
            cosk_sb = cst.tile([P, T], bf16, tag="cosk")
            sink_sb = cst.tile([P, T], bf16, tag="sink")
            deng(nd + 0).dma_start(cosq_sb[:], cosq[:])
            deng(nd + 1).dma_start(sinq_sb[:], sinq[:])
            deng(nd + 2).dma_start(cosk_sb[:], cosk[:])
            deng(nd + 0).dma_start(sink_sb[:], sink[:])
            mask_sb = cst.tile([P, P], bf16, tag="mask")
            deng(nd + 1).dma_start(mask_sb[:], mask1p[:])
            ones_sb = cst.tile([P, 1], bf16, tag="ones")
            deng(nd + 2).dma_start(ones_sb[:], onesp[:])
            nd += 3
            wv_tiles = {}
            for ct in range(16):
                deng(nd).dma_start(x_t[ct][:], xT[ct])
                nd += 1
            for ct in range(16):
                for lp in range(2):
                    wt = wv_pool.tile([P, 512], bf16, tag="wv", name=f"wv{lp}_{ct}")
                    deng(nd).dma_start(wt[:], wv[lp, ct])
                    nd += 1
                    wv_tiles[lp, ct] = wt

            wqk_cm = tc.tile_pool(name="wqkp", bufs=4)
            wqk_pool = wqk_cm.__enter__()
            rope_cm = tc.tile_pool(name="ropep", bufs=4)
            rope_pool = rope_cm.__enter__()

            # ---- Q/K projections (fp8 DoubleRow) with fused RoPE ----
            nr = 0
            for hh in range(8):
                li, dh = hh // 2, hh % 2
                for qk, dest, cos_t, sin_t in (
                    (0, qT_h, cosq_sb, sinq_sb),
                    (1, kT_h, cosk_sb, sink_sb),
                ):
                    wt = wqk_pool.tile([P, 8, 2, P], f8, tag="wqk")
                    deng(nr).dma_start(wt[:], wqk8[qk, li, dh])
                    pps = [
                        ps_proj.tile([P, 512], f32, tag="proj", name=f"pp{qk}_{hh}_{t}")
                        for t in range(2)
                    ]
                    for ce in range(8):
                        for tci in range(2):
                            nc.tensor.matmul(
                                pps[tci][:],
                                wt[:, ce, :, :],
                                x8_t[ce][:, :, tci * 512 : (tci + 1) * 512],
                                start=(ce == 0),
                                stop=(ce == 7),
                                perf_mode=DR,
                            )
                    for tci in range(2):
                        tsl = slice(tci * 512, (tci + 1) * 512)
                        qsb = rope_pool.tile([P, 512], bf16, tag="qsb")
                        copy_op(nr, qsb[:], pps[tci][:])
                        qsw = rope_pool.tile([P, 512], bf16, tag="qsw")
                        deng(nr).dma_start(qsw[0:64, :], qsb[64:128, :])
                        deng(nr + 1).dma_start(qsw[64:128, :], qsb[0:64, :])
                        t1 = rope_pool.tile([P, 512], bf16, tag="t1")
                        t2 = rope_pool.tile([P, 512], bf16, tag="t2")
                        nc.vector.tensor_tensor(t1[:], qsb[:], cos_t[:, tsl], ALU.mult)
                        nc.gpsimd.tensor_tensor(t2[:], qsw[:], sin_t[:, tsl], ALU.mult)
                        nc.vector.tensor_tensor(dest[hh][:, tsl], t1[:], t2[:], ALU.add)
                        nr += 1

            rope_cm.__exit__(None, None, None)
            wqk_cm.__exit__(None, None, None)

            # ---- V projection (resident in SBUF), after QK so the PE can
            # start ~3us in on the small fp8 x ----
            nv = 0
            for lp in range(2):
                for tt in range(8):
                    vps = ps_proj.tile([P, 512], f32, tag="proj")
                    for ct in range(16):
                        nc.tensor.matmul(
                            vps[:],
                            x_t[ct][:, tt * P : (tt + 1) * P],
                            wv_tiles[lp, ct][:],
                            start=(ct == 0),
                            stop=(ct == 15),
                        )
                    copy_op(nv, v_sb[lp, tt][:], vps[:])
                    nv += 1

            wv_cm.__exit__(None, None, None)
            xp_cm.__exit__(None, None, None)

            yp_cm = tc.tile_pool(name="yp", bufs=1)
            yp = yp_cm.__enter__()
            et_cm = tc.tile_pool(name="etpool", bufs=24)
            et_pool = et_cm.__enter__()

            # combined (mixer @ wo) weights, loaded once; opened after the
            # attention pools so their space-reuse waits don't gate exp
            w2_cm = tc.tile_pool(name="w2p", bufs=1)
            w2_pool = w2_cm.__enter__()
            cw_tiles = []
            for ft in range(16):
                cwt = w2_pool.tile([P, 8, P], bf16, tag=f"cw{ft}", name=f"cw{ft}")
                deng(ft).dma_start(cwt[:], cw[ft])
                cw_tiles.append(cwt)

            y_sb = {}
            for qc in range(2):
                y_sb[qc] = yp.tile([P, 8, 512], bf16, tag=f"y{qc}", name=f"ysb{qc}")

            ncp = 0

            def emit_softmaxT(h, qc):
                """S^T chunks + mask + exp -> E^T tiles; returns (ETs, lps)."""
                nkt = 4 * (qc + 1)
                ETs = [
                    et_pool.tile([P, 512], bf16, tag="ET", name=f"ET{h}_{qc}_{ii}")
                    for ii in range(nkt)
                ]
                for kt in range(nkt):
                    fr = max(0, kt - 4 * qc) * P
                    sps = ps_s.tile([P, 512], f32, tag="S", name=f"sps{h}_{qc}_{kt}")
                    nc.tensor.matmul(
                        sps[:, fr:512],
                        kT_h[h][:, kt * P : (kt + 1) * P],
                        qT_h[h][:, qc * 512 + fr : qc * 512 + 512],
                        start=True,
                        stop=True,
                    )
                    nc.scalar.activation(ETs[kt][:, fr:512], sps[:, fr:512], AF.Exp)
                    if kt >= 4 * qc:
                        # zero the strict upper triangle of the diagonal block
                        # (multiplicative 0/1 mask, SBUF-only so gpsimd is ok)
                        nc.gpsimd.tensor_tensor(
                            ETs[kt][:, fr : fr + P],
                            ETs[kt][:, fr : fr + P],
                            mask_sb[:],
                            ALU.mult,
                        )
                return ETs

            def emit_opv(h, qc, ETs):
                """ones-matmul denominator + PV + normalized y copy."""
                nonlocal ncp
                nkt = 4 * (qc + 1)
                lps = ps_l.tile([1, 512], f32, tag="l", name=f"l{h}_{qc}")
                for kt in range(nkt):
                    fr = max(0, kt - 4 * qc) * P
                    nc.tensor.matmul(
                        lps[:, fr:512],
                        ones_sb[:],
                        ETs[kt][:, fr:512],
                        start=(kt == 0),
                        stop=(kt == nkt - 1),
                    )
                rec = rec_pool.tile([1, 512], f32, tag="rec", name=f"rec{h}_{qc}")
                nc.vector.reciprocal(rec[:], lps[:])
                rbc = rec_pool.tile([P, 512], f32, tag="rbc", name=f"rbc{h}_{qc}")
                nc.gpsimd.partition_broadcast(rbc[:], rec[:])
                lp = h // 4
                col = slice((h % 4) * P, (h % 4 + 1) * P)
                yps = ps_y.tile([P, 512], f32, tag="y")
                for kt in range(nkt):
                    fr = max(0, kt - 4 * qc) * P
                    nc.tensor.matmul(
                        yps[:, fr:512],
                        v_sb[lp, kt][:, col],
                        ETs[kt][:, fr:512],
                        start=(kt == 0),
                        stop=(kt == nkt - 1),
                    )
                nc.vector.tensor_tensor(
                    y_sb[qc][:, h, :],
                    yps[:],
                    rbc[:],
                    ALU.mult,
                )

            def emit_wo(qc):
                nonlocal ncp
                tsl = slice(qc * 512, (qc + 1) * 512)
                for ft in range(16):
                    ops = ps_proj.tile([P, 512], f32, tag="proj", name=f"ops{qc}_{ft}")
                    for j in range(8):
                        nc.tensor.matmul(
                            ops[:],
                            cw_tiles[ft][:, j, :],
                            y_sb[qc][:, j, :],
                            start=(j == 0),
                            stop=(j == 7),
                        )
                    osb = stage_pool.tile([P, 512], f32, tag="osb", name=f"osb{qc}_{ft}")
                    copy_op(ncp, osb[:], ops[:])
                    ncp += 1
                    deng(ft).dma_start(outT[ft * P : (ft + 1) * P, tsl], osb[:])

            # ---- attention, software-pipelined (depth 2); wo interleaved ----
            queue = []
            for qc in range(2):
                for h in range(8):
                    ETs = emit_softmaxT(h, qc)
                    queue.append((h, qc, ETs))
                    if len(queue) > 2:
                        ph, pqc, pETs = queue.pop(0)
                        emit_opv(ph, pqc, pETs)
                        if ph == 7 and pqc == 0:
                            emit_wo(0)
            for ph, pqc, pETs in queue:
                emit_opv(ph, pqc, pETs)
                if ph == 7 and pqc == 0:
                    emit_wo(0)
            emit_wo(1)

            w2_cm.__exit__(None, None, None)
            et_cm.__exit__(None, None, None)
            yp_cm.__exit__(None, None, None)
            vsb_cm.__exit__(None, None, None)
            qks_cm.__exit__(None, None, None)

    nc.finalize()
    _NC_CACHE[key] = nc
    return nc


def _run(inputs: dict, trace: bool = False):
    from concourse.bass_utils import run_bass_kernel_spmd

    in_maps = []
    for c in range(NCORES):
        b, g = c // 2, c % 2
        in_maps.append(_prep_core_inputs(inputs, b, g))

    nc = _build_nc()
    res = run_bass_kernel_spmd(nc, in_maps, list(range(NCORES)), trace=trace)

    out = np.empty((B, T, C), dtype=np.float32)
    for b in range(B):
        acc = res.results[2 * b]["outT"] + res.results[2 * b + 1]["outT"]
        out[b] = acc.T
    return out, res


def kernel(**inputs) -> np.ndarray:
    out, _ = _run(inputs, trace=False)
    return out
